# revision 46
# baseline (speedup 1.0000x reference)
"""Multi-head attention TRN2 kernel (B=2, S=2048, D=1024, H=16).

Sharding (8 cores): B(2) x head-group(2) x query-block(2).
Each core: one batch b, 8 heads, 1024 query rows. The output projection
is a per-head-group partial sum; the host adds the two partials while
gathering (unshard step).

Host prep: all inputs are converted to bf16 and pre-folded to
[128 partitions, chunk, ...] contiguous layouts so every on-chip DMA is
one flat transfer, issued on the sync queue in PE-need order (transfers
serialize on the shared SDMA rings, so queue order == arrival order).
The V bias is folded into the output bias on the host
(bo_eff = bo + Wo_g @ bv_g, valid because softmax weights sum to 1).

Softmax: the reference masks scores with -1e-9 (sic), so masked
positions contribute exp(-1e-9) == 1.0f exactly. PV weights are
a = (exp(s/8) - 1) * m, computed post-exp in bf16 so the DVE runs in
2x/4x perf mode (the pre-exp PSUM variant is stuck at 1x); the "+1
everywhere" plane is restored from HOST-precomputed V column sums
(svh = (sum_tok v) @ Wv + S*bv) and an appended ones-column on V gives
the mask-dependent part of the denominator.

Schedule (all aimed at keeping the PE HAM-warm — any PE idle gap over
~3.4us re-throttles the clock from 2.4 to 1.2 GHz):
- per kc chunk: one [128,2,512] score tile (two 64-row head matmuls,
  tile_position-packed to run concurrently), one [128,1024] ACT exp
  straight from PSUM, an in-place (E-1) tensor_scalar at 4x and a
  *pre-duplicated-mask multiply at 2x, two PV matmuls emitted with
  LAG=2 so the PE FIFO always holds independent score work;
- the softmax tail (reciprocal_approx_fast + gpsimd partition_broadcast
  + renormalize) is deferred into the NEXT iteration's emission so its
  DVE/GpSimd ops queue behind that iteration's mask ops;
- pv PSUM banks are evacuated by one ACT copy immediately so the next
  iteration's PV accumulation starts without a stall;
- the V projection and proj_pair(0) interleave with DMA arrivals at
  startup; the nb=0 output projection is emitted inside the last
  attention iteration to overlap the final tails.
"""

import sys

if "/opt/trn_rl_repo" not in sys.path:
    sys.path.insert(0, "/opt/trn_rl_repo")

import numpy as np
import ml_dtypes

import concourse.bass as bass
import concourse.tile as tile
from concourse import bacc, mybir
from concourse.bass_utils import run_bass_kernel_spmd

F32 = mybir.dt.float32
F32R = mybir.dt.float32r
BF16 = mybir.dt.bfloat16
AF = mybir.ActivationFunctionType
ALU = mybir.AluOpType

B, S, D, H = 2, 2048, 1024, 16
DK = 64
Q = 1024          # query rows per core
DH = 512          # head-group feature dims per core
NPAIR = 4         # head pairs per core
KC = S // 128     # 16 contraction chunks over k tokens
EC = D // 128     # 8 contraction chunks over model dim
QNB = Q // 512    # 2 query n-blocks
SNB = S // 512    # 4 khT n-blocks
HC = DH // 128    # 4 xT partition chunks

_PROGRAM = None


def _build_program():
    nc = bacc.Bacc("TRN2", debug=False, num_devices=8)

    # all host-side arrays are pre-folded to [128 partitions, ...] so every
    # DMA is a flat contiguous 2D transfer (minimal descriptor count)
    qT = nc.dram_tensor("qT", [128, EC, Q], BF16, kind="ExternalInput")
    kT = nc.dram_tensor("kT", [128, SNB, EC, 512], BF16, kind="ExternalInput")
    vT = nc.dram_tensor("vT", [128, 4, EC, 512], BF16, kind="ExternalInput")
    maskT = nc.dram_tensor("maskT", [128, KC // 2, QNB, 2, 2, 512], BF16,
                           kind="ExternalInput")
    svhv = nc.dram_tensor("svhv", [65, 8], F32, kind="ExternalInput")
    wqT = nc.dram_tensor("wqT", [128, EC, DH], BF16, kind="ExternalInput")
    wkT = nc.dram_tensor("wkT", [128, EC, DH], BF16, kind="ExternalInput")
    wvT = nc.dram_tensor("wvT", [128, EC, DH], BF16, kind="ExternalInput")
    woT = nc.dram_tensor("woT", [128, HC, D], BF16, kind="ExternalInput")
    bqv = nc.dram_tensor("bqv", [128, NPAIR], F32, kind="ExternalInput")
    bkv = nc.dram_tensor("bkv", [128, NPAIR], F32, kind="ExternalInput")
    bov = nc.dram_tensor("bov", [128, EC], F32, kind="ExternalInput")
    out = nc.dram_tensor("out", [D, Q], F32, kind="ExternalOutput")

    with tile.TileContext(nc) as tc:
        _emit(nc, tc, qT, kT, vT, maskT, wqT, wkT, wvT, woT, bqv, bkv, bov, svhv, out)
    nc.compile()
    return nc


def _emit(nc, tc, qT, kT, vT, maskT, wqT, wkT, wvT, woT, bqv, bkv, bov, svhv, out):
    from contextlib import ExitStack

    ctx = ExitStack()
    with ctx:
        consts = ctx.enter_context(tc.tile_pool(name="consts", bufs=1))
        big = ctx.enter_context(tc.tile_pool(name="big", bufs=1))
        work = ctx.enter_context(tc.tile_pool(name="work", bufs=2))
        pp = ctx.enter_context(tc.tile_pool(name="pp", bufs=2, space="PSUM"))
        pvp = ctx.enter_context(tc.tile_pool(name="pvp", bufs=1, space="PSUM"))
        scp = ctx.enter_context(tc.tile_pool(name="scp", bufs=1, space="PSUM"))
        kqd_cm = tc.tile_pool(name="kqd", bufs=1, side="right")
        kqd = kqd_cm.__enter__()

        # ---- small constants ----
        t_bq = consts.tile([128, NPAIR], F32)
        t_bk = consts.tile([128, NPAIR], F32)
        t_bo = consts.tile([128, EC], F32)
        nc.gpsimd.dma_start(out=t_bq, in_=bqv[:, :])
        nc.gpsimd.dma_start(out=t_bk, in_=bkv[:, :])
        nc.gpsimd.dma_start(out=t_bo, in_=bov[:, :])
        svh_sb = consts.tile([65, 8], F32)
        nc.gpsimd.dma_start(out=svh_sb, in_=svhv[:, :])
        # warm the ACT exp table set (~2.7us load) while ACT is idle at
        # startup instead of on the first real exp of the attention loop
        warm = consts.tile([1, 16], F32)
        nc.vector.memset(warm, 0.0)
        warm2 = consts.tile([1, 16], BF16)
        nc.scalar.activation(out=warm2, in_=warm, func=AF.Exp, scale=1.0)

        # ---- resident activation tensors ----
        # only pairs p and p+1 are ever live: rotate kh/qh through 2 buffers
        khTs, qhTs = {}, {}

        def khT(p):
            if p not in khTs:
                khTs[p] = big.tile([128, S], BF16, tag="khT", bufs=2,
                                   name=f"khT{p}")
            return khTs[p]

        def qhT(p):
            if p not in qhTs:
                qhTs[p] = big.tile([128, Q], BF16, tag="qhT", bufs=2,
                                   name=f"qhT{p}")
            return qhTs[p]
        vh_aug = [big.tile([128, 8, 65], BF16, name=f"vha{i}") for i in range(KC)]
        # mask pre-duplicated per head pair, super-tile major:
        # [st, nb, kc2, h2, 512] so a (st, nb) slice is one flat contiguous
        # [128, 2048] operand for the DVE 2x multiply
        mbf = [big.tile([128, 2, QNB, 2, 2, 512], BF16, name=f"mbf{i}")
               for i in range(4)]
        mst = [mbf[i // 2][:, i % 2] for i in range(KC // 2)]

        vs2 = ctx.enter_context(tc.tile_pool(name="vs", bufs=2))
        vtbp = ctx.enter_context(tc.tile_pool(name="vtb", bufs=1))
        wvf2 = vtbp.tile([128, EC, DH], BF16, name="wvf")
        wvb = [wvf2[:, i, :] for i in range(EC)]

        # ---- k/q weights + inputs: few BIG multi-dim DMAs (queue issue
        # time, ~0.6us per dma_start, was the startup limiter) ----
        kTf = kqd.tile([128, SNB, EC, 512], BF16, name="kTf")
        qTf = kqd.tile([128, EC, Q], BF16, name="qTf")
        wkf = kqd.tile([128, EC, DH], BF16, name="wkf")
        wqf = kqd.tile([128, EC, DH], BF16, name="wqf")
        wkb = [wkf[:, i, :] for i in range(EC)]
        wqb = [wqf[:, i, :] for i in range(EC)]

        # inputs in PE-need order; transfers serialize on the shared SDMA
        # rings, so queue order IS arrival order
        vqs = [vs2.tile([128, EC, 512], BF16, tag="vq", bufs=1, name=f"vq{t}")
               for t in range(4)]
        nc.sync.dma_start(out=wkf, in_=wkT[:, :, :])
        nc.sync.dma_start(out=kTf[:, 0], in_=kT[:, 0])
        nc.sync.dma_start(out=kTf[:, 1], in_=kT[:, 1])
        nc.sync.dma_start(out=wvf2, in_=wvT[:, :, :])
        nc.sync.dma_start(out=vqs[0], in_=vT[:, 0])
        nc.sync.dma_start(out=kTf[:, 2], in_=kT[:, 2])
        nc.sync.dma_start(out=vqs[1], in_=vT[:, 1])
        nc.sync.dma_start(out=kTf[:, 3], in_=kT[:, 3])
        nc.sync.dma_start(out=wqf, in_=wqT[:, :, :])
        nc.sync.dma_start(out=qTf, in_=qT[:, :, :])
        nc.sync.dma_start(out=mbf[0], in_=maskT[:, 0:2])
        nc.sync.dma_start(out=vqs[2], in_=vT[:, 2])
        nc.sync.dma_start(out=vqs[3], in_=vT[:, 3])
        for g in range(1, 4):
            nc.sync.dma_start(out=mbf[g], in_=maskT[:, 2 * g:2 * (g + 1)])

        def proj_tile(p, j):
            if j < SNB:
                sb = j
                ps = pp.tile([128, 512], F32, tag="pp", name=f"khps{p}_{sb}")
                for ec in range(EC):
                    nc.tensor.matmul(
                        ps[:, :], wkb[ec][:, p * 128:(p + 1) * 128],
                        kTf[:, sb, ec, :],
                        start=(ec == 0), stop=(ec == EC - 1))
                nc.vector.tensor_scalar(
                    out=khT(p)[:, sb * 512:(sb + 1) * 512], in0=ps[:, :],
                    scalar1=t_bk[:, p:p + 1], scalar2=None, op0=ALU.add)
            else:
                qb = j - SNB
                ps = pp.tile([128, 512], F32, tag="pp", name=f"qhps{p}_{qb}")
                for ec in range(EC):
                    nc.tensor.matmul(
                        ps[:, :], wqb[ec][:, p * 128:(p + 1) * 128],
                        qTf[:, ec, qb * 512:(qb + 1) * 512],
                        start=(ec == 0), stop=(ec == EC - 1))
                nc.vector.tensor_scalar(
                    out=qhT(p)[:, qb * 512:(qb + 1) * 512], in0=ps[:, :],
                    scalar1=t_bq[:, p:p + 1], scalar2=None, op0=ALU.add)

        def proj_pair(p):
            for j in range(SNB + QNB):
                proj_tile(p, j)



        # ---- vh projection setup (interleaved with first attention pass) ----
        for sc in range(KC):
            nc.vector.memset(vh_aug[sc][:, :, 64:65], 1.0)

        def emit_vproj_qtr(qtr):
            vqf = vqs[qtr]
            vq = [vqf[:, i, :] for i in range(EC)]
            for si in range(4):
                sc = qtr * 4 + si
                ps = pp.tile([128, 512], F32, tag="pp", name=f"vps{sc}")
                for ec in range(EC):
                    nc.tensor.matmul(
                        ps[:, :], vq[ec][:, si * 128:(si + 1) * 128], wvb[ec][:, :],
                        start=(ec == 0), stop=(ec == EC - 1))
                nc.scalar.copy(
                    out=vh_aug[sc][:, :, 0:64],
                    in_=ps.rearrange("p (h d) -> p h d", h=8))

        # startup: proj0 tiles and the first V quarters interleaved in DMA
        # arrival order so the PE never head-of-line blocks on a transfer
        proj_tile(0, 0)
        proj_tile(0, 1)
        emit_vproj_qtr(0)
        proj_tile(0, 2)
        emit_vproj_qtr(1)
        proj_tile(0, 3)
        proj_tile(0, 4)
        proj_tile(0, 5)

        # ---- attention ----
        xT = [big.tile([128, Q], BF16, name=f"xT{i}") for i in range(HC)]
        LAG = 2
        if True:
            pending_tail = [None]

            def run_iteration(p, nb, vproj=False, outp=False, last=False):
                pv0 = pvp.tile([65, 512], F32, tag="pv0", name=f"pv0_{p}{nb}")
                pv1 = pvp.tile([65, 512], F32, tag="pv1", name=f"pv1_{p}{nb}")
                Etiles = {}

                def emit_scores(kc):
                    # 2-kc super-tile: 4 score matmuls fill one 4-bank PSUM
                    # tile, then ONE [128,2048] exp / (E-1) / *mask pass —
                    # halves the per-instruction overhead on ACT and DVE
                    st = kc // 2
                    if kc % 2 == 0:
                        Etiles[('sc', st)] = scp.tile(
                            [128, 2, 2, 512], F32, tag="sc",
                            name=f"sc_{p}_{nb}_{st}")
                    sc_ps = Etiles[('sc', st)]
                    nc.tensor.matmul(
                        sc_ps[:, kc % 2, 0, :],
                        khT(p)[0:64, kc * 128:(kc + 1) * 128],
                        qhT(p)[0:64, nb * 512:(nb + 1) * 512],
                        start=True, stop=True)
                    nc.tensor.matmul(
                        sc_ps[:, kc % 2, 1, :],
                        khT(p)[64:128, kc * 128:(kc + 1) * 128],
                        qhT(p)[64:128, nb * 512:(nb + 1) * 512],
                        start=True, stop=True, tile_position=(64, 0))
                    if kc % 2 == 0:
                        return
                    # exp straight from PSUM, then mask post-exp in bf16:
                    # a = (E - 1) * m runs at DVE 2x/4x (the reference's
                    # -1e-9 masking makes masked weights exactly 1; the +1
                    # plane is restored via the host-computed svh sums)
                    del Etiles[('sc', st)]
                    E = work.tile([128, 2, 2, 512], BF16, tag="E", bufs=2,
                                  name=f"E{p}_{nb}_{st}")
                    Ef = E.rearrange("p k h q -> p (k h q)")
                    nc.scalar.activation(
                        out=Ef, in_=sc_ps.rearrange("p k h q -> p (k h q)"),
                        func=AF.Exp, scale=0.125)
                    nc.vector.tensor_scalar(
                        out=Ef, in0=Ef, scalar1=-1.0, scalar2=None, op0=ALU.add)
                    nc.vector.tensor_tensor(
                        out=Ef, in0=Ef,
                        in1=mst[st][:, nb].rearrange("p k h q -> p (k h q)"),
                        op=ALU.mult)
                    Etiles[st] = E

                def emit_pv(kc):
                    E = Etiles[kc // 2]
                    if kc % 2 == 1:
                        del Etiles[kc // 2]
                    for hh in range(2):
                        nc.tensor.matmul(
                            (pv0, pv1)[hh][:, :], vh_aug[kc][:, 2 * p + hh, :],
                            E[:, kc % 2, hh, :],
                            start=(kc == 0), stop=(kc == KC - 1))

                for kc in range(KC + LAG):
                    if vproj and kc < KC and kc % 8 == 0:
                        emit_vproj_qtr(2 + kc // 8)
                    if outp and kc == 6:
                        emit_outproj(0)
                    if kc < KC:
                        emit_scores(kc)
                    if kc == 3 and pending_tail[0] is not None:
                        # previous iteration's tail drops into the DVE/GpSimd
                        # queues BEHIND this iteration's first mask ops
                        pending_tail[0]()
                        pending_tail[0] = None
                    if kc >= LAG:
                        emit_pv(kc - LAG)

                # evacuate PSUM immediately (ACT) so the pv banks free fast;
                # the rest of the tail is deferred into the next iteration.
                # The final iteration skips the copy (nothing reuses its pv
                # banks) and shortens the drain chain.
                if last:
                    pvns = [pv0, pv1]
                else:
                    pvns = []
                    for hh, pv in ((0, pv0), (1, pv1)):
                        pvn = work.tile([65, 512], F32, tag="pvn", bufs=2,
                                        name=f"pvn{p}{nb}{hh}")
                        if hh == 0:
                            nc.scalar.copy(out=pvn, in_=pv[:, :])
                        else:
                            nc.vector.tensor_copy(out=pvn, in_=pv[:, :])
                        pvns.append(pvn)

                def tail():
                    h0 = 2 * p
                    rcps = []
                    for hh in range(2):
                        den = work.tile([1, 512], F32, tag="den", bufs=1,
                                        name=f"den{p}{nb}{hh}")
                        nc.scalar.activation(
                            out=den, in_=pvns[hh][64:65, :], func=AF.Identity,
                            bias=svh_sb[64:65, h0 + hh:h0 + hh + 1], scale=1.0)
                        rcp_f = work.tile([1, 512], F32, tag="rcpf", bufs=1,
                                          name=f"rcpf{p}{nb}{hh}")
                        nc.vector.reciprocal_approx_fast(out=rcp_f, in_=den)
                        rcps.append(rcp_f)
                    brs = []
                    for hh in range(2):
                        br_sb = work.tile([64, 512], F32, tag="brs", bufs=1,
                                          name=f"brs{p}{nb}{hh}")
                        nc.gpsimd.partition_broadcast(br_sb, rcps[hh])
                        brs.append(br_sb)
                    for hh in range(2):
                        h = 2 * p + hh
                        nc.vector.scalar_tensor_tensor(
                            out=xT[h // 2][(h % 2) * 64:(h % 2) * 64 + 64,
                                           nb * 512:(nb + 1) * 512],
                            in0=pvns[hh][0:64, :], scalar=svh_sb[0:64, h:h + 1],
                            in1=brs[hh], op0=ALU.add, op1=ALU.mult)

                pending_tail[0] = tail

            wob = None
            for p in range(NPAIR):
                for nb in range(QNB):
                    run_iteration(p, nb, vproj=(p == 0 and nb == 0),
                                  outp=(p == 3 and nb == 1),
                                  last=(p == 3 and nb == 1))
                if p + 1 < NPAIR:
                    proj_pair(p + 1)
                if p + 1 == NPAIR - 1:
                    # kq inputs/weights are dead once proj_pair(3) is emitted;
                    # reuse the space for wob so its DMA hides under p=3
                    kqd_cm.__exit__(None, None, None)
                    wop = ctx.enter_context(tc.tile_pool(name="wop", bufs=1))
                    wof = wop.tile([128, HC, D], BF16, name="wof")
                    wob = [wof[:, i, :] for i in range(HC)]
                    nc.sync.dma_start(out=wof, in_=woT[:, :, :])
            def emit_outproj(nb):
                for dc in range(EC):
                    ps = pp.tile([128, 512], F32, tag="pp", name=f"ops{dc}_{nb}")
                    for hc in range(HC):
                        nc.tensor.matmul(
                            ps[:, :], wob[hc][:, dc * 128:(dc + 1) * 128],
                            xT[hc][:, nb * 512:(nb + 1) * 512],
                            start=(hc == 0), stop=(hc == HC - 1))
                    o_sb = work.tile([128, 512], F32, tag="osb", bufs=2,
                                     name=f"osb{dc}_{nb}")
                    nc.scalar.add(out=o_sb, in_=ps[:, :], add=t_bo[:, dc:dc + 1])
                    nc.sync.dma_start(
                        out=out[dc * 128:(dc + 1) * 128, nb * 512:(nb + 1) * 512],
                        in_=o_sb)

            pending_tail[0]()
            pending_tail[0] = None
            emit_outproj(1)


def _get_program():
    global _PROGRAM
    if _PROGRAM is None:
        _PROGRAM = _build_program()
    return _PROGRAM


def kernel(q, k, v, mask, Wq, bq, Wk, bk, Wv, bv, Wo, bo, _trace=False):
    bf16 = ml_dtypes.bfloat16
    q = np.asarray(q, np.float32)
    k = np.asarray(k, np.float32)
    v = np.asarray(v, np.float32)
    Wq = np.asarray(Wq, np.float32)
    Wk = np.asarray(Wk, np.float32)
    Wv = np.asarray(Wv, np.float32)
    Wo = np.asarray(Wo, np.float32)
    bq = np.asarray(bq, np.float32)
    bk = np.asarray(bk, np.float32)
    bv = np.asarray(bv, np.float32)
    bo = np.asarray(bo, np.float32)
    mask_f = np.asarray(mask).astype(np.float32)

    nc = _get_program()

    # fold the D (or S) axis into [128 partitions, chunk, ...] so every
    # on-chip DMA is one flat contiguous transfer
    def fold_ec(xT, inner):  # [D, N] -> [128, EC, N] (or [128, x, y] views)
        N = xT.shape[1]
        a = np.ascontiguousarray(xT.reshape(EC, 128, N).transpose(1, 0, 2))
        return a.astype(bf16).reshape((128,) + inner)

    # kT: [D, S] -> [128, SNB, EC, 512] (512-col block major)
    kT_b = [np.ascontiguousarray(
        k[b].T.reshape(EC, 128, SNB, 512).transpose(1, 2, 0, 3)).astype(bf16)
        for b in range(B)]
    # vT: [D, S] -> [128, 4, EC, 512] (quarter major)
    vT_b = [np.ascontiguousarray(
        v[b].T.reshape(EC, 128, 4, 512).transpose(1, 2, 0, 3)).astype(bf16)
        for b in range(B)]
    # mask duplicated per head pair, super-tile major:
    # [128, KC//2, QNB, kc2, h2, 512]
    def build_mdup(mT):  # mT: [S, Q] 0/1 float
        a = mT.reshape(KC // 2, 2, 128, QNB, 512).transpose(2, 0, 3, 1, 4)
        a = np.broadcast_to(a[:, :, :, :, None, :],
                            (128, KC // 2, QNB, 2, 2, 512))
        return np.ascontiguousarray(a).astype(bf16)
    mdup_b = [[build_mdup(mask_f[b, 0, sq * Q:(sq + 1) * Q, :].T)
               for sq in range(2)] for b in range(B)]
    # host-side V column sums: svh[0:64, h] = (sum_tok v) @ Wv_h + S*bv_h
    svh_b = []
    for b in range(B):
        vsum = v[b].sum(0)  # [D]
        per_g = []
        for hg in range(2):
            hsl_g = slice(hg * DH, (hg + 1) * DH)
            s = vsum @ Wv[hsl_g, :].T + S * bv[hsl_g]  # [DH]
            arr = np.zeros((65, 8), np.float32)
            arr[0:64, :] = s.reshape(8, 64).T
            arr[64, :] = S
            per_g.append(np.ascontiguousarray(arr))
        svh_b.append(per_g)
    wqT_f = np.ascontiguousarray(Wq.T).astype(np.float32)
    wkT_f = np.ascontiguousarray(Wk.T)
    wvT_f = np.ascontiguousarray(Wv.T)

    in_maps = []
    for c in range(8):
        b, hg, sq = c // 4, (c // 2) % 2, c % 2
        hsl = slice(hg * DH, (hg + 1) * DH)
        in_maps.append({
            "qT": fold_ec(q[b, sq * Q:(sq + 1) * Q, :].T, (EC, Q)),
            "kT": kT_b[b],
            "vT": vT_b[b],
            "maskT": mdup_b[b][sq],
            "svhv": svh_b[b][hg],
            "wqT": fold_ec(Wq.T[:, hsl], (EC, DH)),
            "wkT": fold_ec(Wk.T[:, hsl], (EC, DH)),
            "wvT": fold_ec(Wv.T[:, hsl], (EC, DH)),
            "woT": np.ascontiguousarray(
                Wo.T[hsl, :].reshape(HC, 128, D).transpose(1, 0, 2)).astype(bf16),
            "bqv": np.ascontiguousarray(bq[hsl].reshape(NPAIR, 128).T),
            "bkv": np.ascontiguousarray(bk[hsl].reshape(NPAIR, 128).T),
            "bov": np.ascontiguousarray(
                ((bo if hg == 0 else np.zeros_like(bo))
                 + Wo[:, hsl] @ bv[hsl]).reshape(EC, 128).T),
        })

    kw = {}
    if _trace:
        kw = dict(trace=True, trace_cores=list(range(8)))
    res = run_bass_kernel_spmd(nc, in_maps, core_ids=list(range(8)), **kw)
    kernel._last_res = res

    outp = np.empty((B, S, D), np.float32)
    for b in range(B):
        for sq in range(2):
            c0 = b * 4 + sq
            c1 = b * 4 + 2 + sq
            outp[b, sq * Q:(sq + 1) * Q, :] = (
                res.results[c0]["out"] + res.results[c1]["out"]).T
    if _trace:
        return outp, res
    return outp


# revision 47
# speedup vs baseline: 1.0582x; 1.0582x over previous
"""Multi-head attention TRN2 kernel (B=2, S=2048, D=1024, H=16).

Sharding (8 cores): B(2) x head-group(2) x query-block(2).
Each core: one batch b, 8 heads, 1024 query rows. The output projection
is a per-head-group partial sum; the host adds the two partials while
gathering (unshard step).

Host prep: all inputs are converted to bf16 and pre-folded to
[128 partitions, chunk, ...] contiguous layouts so every on-chip DMA is
one flat transfer, issued on the sync queue in PE-need order (transfers
serialize on the shared SDMA rings, so queue order == arrival order).
The V bias is folded into the output bias on the host
(bo_eff = bo + Wo_g @ bv_g, valid because softmax weights sum to 1).

Softmax: the reference masks scores with -1e-9 (sic), so masked
positions contribute exp(-1e-9) == 1.0f exactly. PV weights are
a = (exp(s/8) - 1) * m, computed post-exp in bf16 so the DVE runs in
2x/4x perf mode (the pre-exp PSUM variant is stuck at 1x); the "+1
everywhere" plane is restored from HOST-precomputed V column sums
(svh = (sum_tok v) @ Wv + S*bv) and an appended ones-column on V gives
the mask-dependent part of the denominator.

Schedule (all aimed at keeping the PE HAM-warm — any PE idle gap over
~3.4us re-throttles the clock from 2.4 to 1.2 GHz):
- per kc chunk: one [128,2,512] score tile (two 64-row head matmuls,
  tile_position-packed to run concurrently), one [128,1024] ACT exp
  straight from PSUM, an in-place (E-1) tensor_scalar at 4x and a
  *pre-duplicated-mask multiply at 2x, two PV matmuls emitted with
  LAG=2 so the PE FIFO always holds independent score work;
- the softmax tail (reciprocal_approx_fast + gpsimd partition_broadcast
  + renormalize) is deferred into the NEXT iteration's emission so its
  DVE/GpSimd ops queue behind that iteration's mask ops;
- pv PSUM banks are evacuated by one ACT copy immediately so the next
  iteration's PV accumulation starts without a stall;
- the V projection and proj_pair(0) interleave with DMA arrivals at
  startup; the nb=0 output projection is emitted inside the last
  attention iteration to overlap the final tails.
"""

import sys

if "/opt/trn_rl_repo" not in sys.path:
    sys.path.insert(0, "/opt/trn_rl_repo")

import numpy as np
import ml_dtypes

import concourse.bass as bass
import concourse.tile as tile
from concourse import bacc, mybir
from concourse.bass_utils import run_bass_kernel_spmd

F32 = mybir.dt.float32
F32R = mybir.dt.float32r
BF16 = mybir.dt.bfloat16
AF = mybir.ActivationFunctionType
ALU = mybir.AluOpType

B, S, D, H = 2, 2048, 1024, 16
DK = 64
Q = 1024          # query rows per core
DH = 512          # head-group feature dims per core
NPAIR = 4         # head pairs per core
KC = S // 128     # 16 contraction chunks over k tokens
EC = D // 128     # 8 contraction chunks over model dim
QNB = Q // 512    # 2 query n-blocks
SNB = S // 512    # 4 khT n-blocks
HC = DH // 128    # 4 xT partition chunks

_PROGRAM = None


def _build_program():
    nc = bacc.Bacc("TRN2", debug=False, num_devices=8)

    # all host-side arrays are pre-folded to [128 partitions, ...] so every
    # DMA is a flat contiguous 2D transfer (minimal descriptor count)
    qT = nc.dram_tensor("qT", [128, EC, Q], BF16, kind="ExternalInput")
    kT = nc.dram_tensor("kT", [128, SNB, EC, 512], BF16, kind="ExternalInput")
    vT = nc.dram_tensor("vT", [128, 4, EC, 512], BF16, kind="ExternalInput")
    maskT = nc.dram_tensor("maskT", [128, KC, QNB, 2, 512], BF16,
                           kind="ExternalInput")
    svhv = nc.dram_tensor("svhv", [65, 8], F32, kind="ExternalInput")
    wqT = nc.dram_tensor("wqT", [128, EC, DH], BF16, kind="ExternalInput")
    wkT = nc.dram_tensor("wkT", [128, EC, DH], BF16, kind="ExternalInput")
    wvT = nc.dram_tensor("wvT", [128, EC, DH], BF16, kind="ExternalInput")
    woT = nc.dram_tensor("woT", [128, HC, D], BF16, kind="ExternalInput")
    bqv = nc.dram_tensor("bqv", [128, NPAIR], F32, kind="ExternalInput")
    bkv = nc.dram_tensor("bkv", [128, NPAIR], F32, kind="ExternalInput")
    bov = nc.dram_tensor("bov", [128, EC], F32, kind="ExternalInput")
    out = nc.dram_tensor("out", [D, Q], F32, kind="ExternalOutput")

    with tile.TileContext(nc) as tc:
        _emit(nc, tc, qT, kT, vT, maskT, wqT, wkT, wvT, woT, bqv, bkv, bov, svhv, out)
    nc.compile()
    return nc


def _emit(nc, tc, qT, kT, vT, maskT, wqT, wkT, wvT, woT, bqv, bkv, bov, svhv, out):
    from contextlib import ExitStack

    ctx = ExitStack()
    with ctx:
        consts = ctx.enter_context(tc.tile_pool(name="consts", bufs=1))
        big = ctx.enter_context(tc.tile_pool(name="big", bufs=1))
        work = ctx.enter_context(tc.tile_pool(name="work", bufs=2))
        pp = ctx.enter_context(tc.tile_pool(name="pp", bufs=2, space="PSUM"))
        pvp = ctx.enter_context(tc.tile_pool(name="pvp", bufs=1, space="PSUM"))
        scp = ctx.enter_context(tc.tile_pool(name="scp", bufs=2, space="PSUM"))
        kqd_cm = tc.tile_pool(name="kqd", bufs=1, side="right")
        kqd = kqd_cm.__enter__()

        # ---- small constants ----
        t_bq = consts.tile([128, NPAIR], F32)
        t_bk = consts.tile([128, NPAIR], F32)
        t_bo = consts.tile([128, EC], F32)
        nc.gpsimd.dma_start(out=t_bq, in_=bqv[:, :])
        nc.gpsimd.dma_start(out=t_bk, in_=bkv[:, :])
        nc.gpsimd.dma_start(out=t_bo, in_=bov[:, :])
        svh_sb = consts.tile([65, 8], F32)
        nc.gpsimd.dma_start(out=svh_sb, in_=svhv[:, :])
        # warm the ACT exp table set (~2.7us load) while ACT is idle at
        # startup instead of on the first real exp of the attention loop
        warm = consts.tile([1, 16], F32)
        nc.vector.memset(warm, 0.0)
        warm2 = consts.tile([1, 16], BF16)
        nc.scalar.activation(out=warm2, in_=warm, func=AF.Exp, scale=1.0)

        # ---- resident activation tensors ----
        # only pairs p and p+1 are ever live: rotate kh/qh through 2 buffers
        khTs, qhTs = {}, {}

        def khT(p):
            if p not in khTs:
                khTs[p] = big.tile([128, S], BF16, tag="khT", bufs=2,
                                   name=f"khT{p}")
            return khTs[p]

        def qhT(p):
            if p not in qhTs:
                qhTs[p] = big.tile([128, Q], BF16, tag="qhT", bufs=2,
                                   name=f"qhT{p}")
            return qhTs[p]
        vh_aug = [big.tile([128, 8, 65], BF16, name=f"vha{i}") for i in range(KC)]
        # mask pre-duplicated per head pair: [kc, nb, h2, 512] so the
        # post-exp multiply is one flat [128,1024] bf16 op in DVE 2x mode
        mbf = [big.tile([128, 4, QNB, 2, 512], BF16, name=f"mbf{i}")
               for i in range(4)]
        mb = [mbf[i // 4][:, i % 4] for i in range(KC)]

        vs2 = ctx.enter_context(tc.tile_pool(name="vs", bufs=2))
        vtbp = ctx.enter_context(tc.tile_pool(name="vtb", bufs=1))
        wvf2 = vtbp.tile([128, EC, DH], BF16, name="wvf")
        wvb = [wvf2[:, i, :] for i in range(EC)]

        # ---- k/q weights + inputs: few BIG multi-dim DMAs (queue issue
        # time, ~0.6us per dma_start, was the startup limiter) ----
        kTf = kqd.tile([128, SNB, EC, 512], BF16, name="kTf")
        qTf = kqd.tile([128, EC, Q], BF16, name="qTf")
        wkf = kqd.tile([128, EC, DH], BF16, name="wkf")
        wqf = kqd.tile([128, EC, DH], BF16, name="wqf")
        wkb = [wkf[:, i, :] for i in range(EC)]
        wqb = [wqf[:, i, :] for i in range(EC)]

        # inputs in PE-need order; transfers serialize on the shared SDMA
        # rings, so queue order IS arrival order
        vqs = [vs2.tile([128, EC, 512], BF16, tag="vq", bufs=1, name=f"vq{t}")
               for t in range(4)]
        nc.sync.dma_start(out=wkf, in_=wkT[:, :, :])
        nc.sync.dma_start(out=kTf[:, 0], in_=kT[:, 0])
        nc.sync.dma_start(out=kTf[:, 1], in_=kT[:, 1])
        nc.sync.dma_start(out=wvf2, in_=wvT[:, :, :])
        nc.sync.dma_start(out=vqs[0], in_=vT[:, 0])
        nc.sync.dma_start(out=kTf[:, 2], in_=kT[:, 2])
        nc.sync.dma_start(out=vqs[1], in_=vT[:, 1])
        nc.sync.dma_start(out=kTf[:, 3], in_=kT[:, 3])
        nc.sync.dma_start(out=wqf, in_=wqT[:, :, :])
        nc.sync.dma_start(out=qTf, in_=qT[:, :, :])
        nc.sync.dma_start(out=mbf[0], in_=maskT[:, 0:4])
        nc.sync.dma_start(out=vqs[2], in_=vT[:, 2])
        nc.sync.dma_start(out=vqs[3], in_=vT[:, 3])
        for g in range(1, 4):
            nc.sync.dma_start(out=mbf[g], in_=maskT[:, 4 * g:4 * (g + 1)])

        def proj_tile(p, j):
            if j < SNB:
                sb = j
                ps = pp.tile([128, 512], F32, tag="pp", name=f"khps{p}_{sb}")
                for ec in range(EC):
                    nc.tensor.matmul(
                        ps[:, :], wkb[ec][:, p * 128:(p + 1) * 128],
                        kTf[:, sb, ec, :],
                        start=(ec == 0), stop=(ec == EC - 1))
                nc.vector.tensor_scalar(
                    out=khT(p)[:, sb * 512:(sb + 1) * 512], in0=ps[:, :],
                    scalar1=t_bk[:, p:p + 1], scalar2=None, op0=ALU.add)
            else:
                qb = j - SNB
                ps = pp.tile([128, 512], F32, tag="pp", name=f"qhps{p}_{qb}")
                for ec in range(EC):
                    nc.tensor.matmul(
                        ps[:, :], wqb[ec][:, p * 128:(p + 1) * 128],
                        qTf[:, ec, qb * 512:(qb + 1) * 512],
                        start=(ec == 0), stop=(ec == EC - 1))
                nc.vector.tensor_scalar(
                    out=qhT(p)[:, qb * 512:(qb + 1) * 512], in0=ps[:, :],
                    scalar1=t_bq[:, p:p + 1], scalar2=None, op0=ALU.add)

        def proj_pair(p):
            for j in range(SNB + QNB):
                proj_tile(p, j)



        # ---- vh projection setup (interleaved with first attention pass) ----
        for sc in range(KC):
            nc.vector.memset(vh_aug[sc][:, :, 64:65], 1.0)

        def emit_vproj_qtr(qtr):
            vqf = vqs[qtr]
            vq = [vqf[:, i, :] for i in range(EC)]
            for si in range(4):
                sc = qtr * 4 + si
                ps = pp.tile([128, 512], F32, tag="pp", name=f"vps{sc}")
                for ec in range(EC):
                    nc.tensor.matmul(
                        ps[:, :], vq[ec][:, si * 128:(si + 1) * 128], wvb[ec][:, :],
                        start=(ec == 0), stop=(ec == EC - 1))
                nc.scalar.copy(
                    out=vh_aug[sc][:, :, 0:64],
                    in_=ps.rearrange("p (h d) -> p h d", h=8))

        # startup: proj0 tiles and the first V quarters interleaved in DMA
        # arrival order so the PE never head-of-line blocks on a transfer
        proj_tile(0, 0)
        proj_tile(0, 1)
        emit_vproj_qtr(0)
        proj_tile(0, 2)
        emit_vproj_qtr(1)
        proj_tile(0, 3)
        proj_tile(0, 4)
        proj_tile(0, 5)

        # ---- attention ----
        xT = [big.tile([128, Q], BF16, name=f"xT{i}") for i in range(HC)]
        LAG = 2
        if True:
            pending_tail = [None]

            def run_iteration(p, nb, vproj=False, outp=False, last=False):
                pv0 = pvp.tile([65, 512], F32, tag="pv0", name=f"pv0_{p}{nb}")
                pv1 = pvp.tile([65, 512], F32, tag="pv1", name=f"pv1_{p}{nb}")
                Etiles = {}

                def emit_scores(kc):
                    sc_ps = scp.tile([128, 2, 512], F32, tag="sc",
                                     name=f"sc_{p}_{nb}_{kc}")
                    nc.tensor.matmul(
                        sc_ps[:, 0, :],
                        khT(p)[0:64, kc * 128:(kc + 1) * 128],
                        qhT(p)[0:64, nb * 512:(nb + 1) * 512],
                        start=True, stop=True)
                    nc.tensor.matmul(
                        sc_ps[:, 1, :],
                        khT(p)[64:128, kc * 128:(kc + 1) * 128],
                        qhT(p)[64:128, nb * 512:(nb + 1) * 512],
                        start=True, stop=True, tile_position=(64, 0))
                    # exp straight from PSUM, then mask post-exp in bf16:
                    # a = (E - 1) * m runs at DVE 2x/4x (the reference's
                    # -1e-9 masking makes masked weights exactly 1; the +1
                    # plane is restored via the host-computed svh sums)
                    E = work.tile([128, 2, 512], BF16, tag="E", bufs=3,
                                  name=f"E{p}_{nb}_{kc}")
                    Ef = E.rearrange("p h q -> p (h q)")
                    nc.scalar.activation(
                        out=Ef, in_=sc_ps.rearrange("p h q -> p (h q)"),
                        func=AF.Exp, scale=0.125)
                    nc.vector.tensor_scalar(
                        out=Ef, in0=Ef, scalar1=-1.0, scalar2=None, op0=ALU.add)
                    nc.vector.tensor_tensor(
                        out=Ef, in0=Ef,
                        in1=mb[kc][:, nb].rearrange("p h q -> p (h q)"),
                        op=ALU.mult)
                    Etiles[kc] = E

                def emit_pv(kc):
                    E = Etiles.pop(kc)
                    for hh in range(2):
                        nc.tensor.matmul(
                            (pv0, pv1)[hh][:, :], vh_aug[kc][:, 2 * p + hh, :],
                            E[:, hh, :],
                            start=(kc == 0), stop=(kc == KC - 1))

                for kc in range(KC + LAG):
                    if vproj and kc < KC and kc % 8 == 0:
                        emit_vproj_qtr(2 + kc // 8)
                    if outp and kc == 6:
                        emit_outproj(0)
                    if kc < KC:
                        emit_scores(kc)
                    if kc == 3 and pending_tail[0] is not None:
                        # previous iteration's tail drops into the DVE/GpSimd
                        # queues BEHIND this iteration's first mask ops
                        pending_tail[0]()
                        pending_tail[0] = None
                    if kc >= LAG:
                        emit_pv(kc - LAG)

                # evacuate PSUM immediately (ACT) so the pv banks free fast;
                # the rest of the tail is deferred into the next iteration.
                # The final iteration skips the copy (nothing reuses its pv
                # banks) and shortens the drain chain.
                if last:
                    pvns = [pv0, pv1]
                else:
                    pvns = []
                    for hh, pv in ((0, pv0), (1, pv1)):
                        pvn = work.tile([65, 512], F32, tag="pvn", bufs=2,
                                        name=f"pvn{p}{nb}{hh}")
                        if hh == 0:
                            nc.scalar.copy(out=pvn, in_=pv[:, :])
                        else:
                            nc.vector.tensor_copy(out=pvn, in_=pv[:, :])
                        pvns.append(pvn)

                def tail():
                    h0 = 2 * p
                    rcps = []
                    for hh in range(2):
                        den = work.tile([1, 512], F32, tag="den", bufs=1,
                                        name=f"den{p}{nb}{hh}")
                        nc.scalar.activation(
                            out=den, in_=pvns[hh][64:65, :], func=AF.Identity,
                            bias=svh_sb[64:65, h0 + hh:h0 + hh + 1], scale=1.0)
                        rcp_f = work.tile([1, 512], F32, tag="rcpf", bufs=1,
                                          name=f"rcpf{p}{nb}{hh}")
                        nc.vector.reciprocal_approx_fast(out=rcp_f, in_=den)
                        rcps.append(rcp_f)
                    brs = []
                    for hh in range(2):
                        br_sb = work.tile([64, 512], F32, tag="brs", bufs=1,
                                          name=f"brs{p}{nb}{hh}")
                        nc.gpsimd.partition_broadcast(br_sb, rcps[hh])
                        brs.append(br_sb)
                    for hh in range(2):
                        h = 2 * p + hh
                        nc.vector.scalar_tensor_tensor(
                            out=xT[h // 2][(h % 2) * 64:(h % 2) * 64 + 64,
                                           nb * 512:(nb + 1) * 512],
                            in0=pvns[hh][0:64, :], scalar=svh_sb[0:64, h:h + 1],
                            in1=brs[hh], op0=ALU.add, op1=ALU.mult)

                pending_tail[0] = tail

            wob = None
            for p in range(NPAIR):
                for nb in range(QNB):
                    run_iteration(p, nb, vproj=(p == 0 and nb == 0),
                                  outp=(p == 3 and nb == 1),
                                  last=(p == 3 and nb == 1))
                if p + 1 < NPAIR:
                    proj_pair(p + 1)
                if p + 1 == NPAIR - 1:
                    # kq inputs/weights are dead once proj_pair(3) is emitted;
                    # reuse the space for wob so its DMA hides under p=3
                    kqd_cm.__exit__(None, None, None)
                    wop = ctx.enter_context(tc.tile_pool(name="wop", bufs=1))
                    wof = wop.tile([128, HC, D], BF16, name="wof")
                    wob = [wof[:, i, :] for i in range(HC)]
                    nc.sync.dma_start(out=wof, in_=woT[:, :, :])
            def emit_outproj(nb):
                for dc in range(EC):
                    ps = pp.tile([128, 512], F32, tag="pp", name=f"ops{dc}_{nb}")
                    for hc in range(HC):
                        nc.tensor.matmul(
                            ps[:, :], wob[hc][:, dc * 128:(dc + 1) * 128],
                            xT[hc][:, nb * 512:(nb + 1) * 512],
                            start=(hc == 0), stop=(hc == HC - 1))
                    o_sb = work.tile([128, 512], F32, tag="osb", bufs=2,
                                     name=f"osb{dc}_{nb}")
                    nc.scalar.add(out=o_sb, in_=ps[:, :], add=t_bo[:, dc:dc + 1])
                    nc.sync.dma_start(
                        out=out[dc * 128:(dc + 1) * 128, nb * 512:(nb + 1) * 512],
                        in_=o_sb)

            pending_tail[0]()
            pending_tail[0] = None
            emit_outproj(1)


def _get_program():
    global _PROGRAM
    if _PROGRAM is None:
        _PROGRAM = _build_program()
    return _PROGRAM


def kernel(q, k, v, mask, Wq, bq, Wk, bk, Wv, bv, Wo, bo, _trace=False):
    bf16 = ml_dtypes.bfloat16
    q = np.asarray(q, np.float32)
    k = np.asarray(k, np.float32)
    v = np.asarray(v, np.float32)
    Wq = np.asarray(Wq, np.float32)
    Wk = np.asarray(Wk, np.float32)
    Wv = np.asarray(Wv, np.float32)
    Wo = np.asarray(Wo, np.float32)
    bq = np.asarray(bq, np.float32)
    bk = np.asarray(bk, np.float32)
    bv = np.asarray(bv, np.float32)
    bo = np.asarray(bo, np.float32)
    mask_f = np.asarray(mask).astype(np.float32)

    nc = _get_program()

    # fold the D (or S) axis into [128 partitions, chunk, ...] so every
    # on-chip DMA is one flat contiguous transfer
    def fold_ec(xT, inner):  # [D, N] -> [128, EC, N] (or [128, x, y] views)
        N = xT.shape[1]
        a = np.ascontiguousarray(xT.reshape(EC, 128, N).transpose(1, 0, 2))
        return a.astype(bf16).reshape((128,) + inner)

    # kT: [D, S] -> [128, SNB, EC, 512] (512-col block major)
    kT_b = [np.ascontiguousarray(
        k[b].T.reshape(EC, 128, SNB, 512).transpose(1, 2, 0, 3)).astype(bf16)
        for b in range(B)]
    # vT: [D, S] -> [128, 4, EC, 512] (quarter major)
    vT_b = [np.ascontiguousarray(
        v[b].T.reshape(EC, 128, 4, 512).transpose(1, 2, 0, 3)).astype(bf16)
        for b in range(B)]
    # mask duplicated per head pair: [128, KC, QNB, 2, 512]
    def build_mdup(mT):  # mT: [S, Q] 0/1 float
        a = mT.reshape(KC, 128, QNB, 512).transpose(1, 0, 2, 3)
        a = np.broadcast_to(a[:, :, :, None, :], (128, KC, QNB, 2, 512))
        return np.ascontiguousarray(a).astype(bf16)
    mdup_b = [[build_mdup(mask_f[b, 0, sq * Q:(sq + 1) * Q, :].T)
               for sq in range(2)] for b in range(B)]
    # host-side V column sums: svh[0:64, h] = (sum_tok v) @ Wv_h + S*bv_h
    svh_b = []
    for b in range(B):
        vsum = v[b].sum(0)  # [D]
        per_g = []
        for hg in range(2):
            hsl_g = slice(hg * DH, (hg + 1) * DH)
            s = vsum @ Wv[hsl_g, :].T + S * bv[hsl_g]  # [DH]
            arr = np.zeros((65, 8), np.float32)
            arr[0:64, :] = s.reshape(8, 64).T
            arr[64, :] = S
            per_g.append(np.ascontiguousarray(arr))
        svh_b.append(per_g)
    wqT_f = np.ascontiguousarray(Wq.T).astype(np.float32)
    wkT_f = np.ascontiguousarray(Wk.T)
    wvT_f = np.ascontiguousarray(Wv.T)

    in_maps = []
    for c in range(8):
        b, hg, sq = c // 4, (c // 2) % 2, c % 2
        hsl = slice(hg * DH, (hg + 1) * DH)
        in_maps.append({
            "qT": fold_ec(q[b, sq * Q:(sq + 1) * Q, :].T, (EC, Q)),
            "kT": kT_b[b],
            "vT": vT_b[b],
            "maskT": mdup_b[b][sq],
            "svhv": svh_b[b][hg],
            "wqT": fold_ec(Wq.T[:, hsl], (EC, DH)),
            "wkT": fold_ec(Wk.T[:, hsl], (EC, DH)),
            "wvT": fold_ec(Wv.T[:, hsl], (EC, DH)),
            "woT": np.ascontiguousarray(
                Wo.T[hsl, :].reshape(HC, 128, D).transpose(1, 0, 2)).astype(bf16),
            "bqv": np.ascontiguousarray(bq[hsl].reshape(NPAIR, 128).T),
            "bkv": np.ascontiguousarray(bk[hsl].reshape(NPAIR, 128).T),
            "bov": np.ascontiguousarray(
                ((bo if hg == 0 else np.zeros_like(bo))
                 + Wo[:, hsl] @ bv[hsl]).reshape(EC, 128).T),
        })

    kw = {}
    if _trace:
        kw = dict(trace=True, trace_cores=list(range(8)))
    res = run_bass_kernel_spmd(nc, in_maps, core_ids=list(range(8)), **kw)
    kernel._last_res = res

    outp = np.empty((B, S, D), np.float32)
    for b in range(B):
        for sq in range(2):
            c0 = b * 4 + sq
            c1 = b * 4 + 2 + sq
            outp[b, sq * Q:(sq + 1) * Q, :] = (
                res.results[c0]["out"] + res.results[c1]["out"]).T
    if _trace:
        return outp, res
    return outp
